# revision 39
# baseline (speedup 1.0000x reference)
"""Bilateral filter (nn_BilateralFilter) on 8 Trainium2 NeuronCores.

Sharding (tap-balanced): the 4 samples split into two k=5 samples (25
active taps) and two k=3 samples (9 taps). Each core runs two 16-channel
units -- unit A: (k5 sample, H-half, channel-group) with 25 taps, unit
B: (k3 sample, H-half, channel-group) with 9 taps -- so every core gets
an identical SPMD instruction stream and an equal 34-tap load (vs 25
tap-slots x 32ch under naive (sample, half) sharding where k3 cores
waste 16 zero-weight slots). The host rolls each unit's own 16 channels
to the front of its slab; all 32 channels ship for the channel-mean.
Falls back to uniform union-tap sharding if the k5/k3 split is not 2/2.

Math (exact rewrite of the reference):
  out[c,p] = sum_t W_t[p] x[c,p+t] / (sum_t W_t[p] + 1e-8*n_active)
  W_t[p]   = exp(-((m*s)[p+t] - (m*s)[p])^2 + ln sk_t),
  s = 1/(sqrt(2)*sigma2*C), sk = mask-folded normalized spatial kernel.
The 1e-8*n_active term reproduces the reference's w/(w.sum()+1e-8)
epsilon after multiplying through by the color-kernel normalizer
(sum_t e_t ~= n_active since e_t in [0.96, 1]; abs err <= 1e-8*25*0.04).
Inactive union taps get ln_sk = -100 so their weight underflows to 0.

Engine split: tap multiplies in bf16 (DVE 2x mode) as dj-paired DVE ops
plus GPSIMD single-tap ops; tap accumulation via bf16 identity matmuls
into PSUM on PE (1 cyc/row, 4 groups of 8ch with bank ping-pong);
weights on ACT as one Exp(-d^2 + ln sk) per tap (bf16 out); denominator
as a GPSIMD add chain over W planes (GPSIMD cannot touch PSUM on HW);
output drain: ACT copies PSUM -> bf16 SBUF (freeing banks early for the
next unit), GPSIMD does the recip-divide from SBUF, keeping the entire
output path off the busy DVE. Row-shifted x slabs are reloaded from a bf16
restage of the slab in DRAM (cheap flat DMAs; avoids per-partition-fat
SBUF tail copies). Unit A's channel-mean loads its mean-only chunks
first and sums them with fp32 matmuls on the otherwise-idle PE (no
convert dependency); unit B's channel-mean uses DVE tensor_reduce so it
needs no PSUM bank while unit A's phase B owns all 8 banks.
"""

import numpy as np

B, C, H, W = 4, 32, 256, 256
HALF = H // 2          # output rows per core
SLAB_H = HALF + 4      # input rows incl. 2-row halos
SLAB_W = W + 4         # input cols incl. 2-col halos
NCORES = 8
NT = 25                # 5x5 taps
HC = C // 2            # channels per PSUM group
CHK = 4                # channels per fp32 staging chunk

_CACHE = {}


def _host_tap_constants(params):
    """Per-sample ln_sk[25], n_active, s2c scalar (all float32 math)."""
    p = params.astype(np.float32)
    sig = (1.0 / (1.0 + np.exp(-p))).astype(np.float32)
    coords = (np.arange(5, dtype=np.float32) - 2.0)
    grid = coords[:, None] ** 2 + coords[None, :] ** 2
    center3 = ((np.abs(coords)[:, None] <= 1) & (np.abs(coords)[None, :] <= 1)).astype(np.float32)
    out = []
    for b in range(B):
        k_raw = np.float32(1.0) + np.float32(2.0) * sig[b, 0]
        is5 = bool(k_raw >= 2.0)
        sigma1 = np.float32(3.5) + np.float32(5.5) * sig[b, 1]
        sigma2 = np.float32(5.5) + np.float32(7.5) * sig[b, 2]
        mask = np.ones((5, 5), np.float32) if is5 else center3
        sk = np.exp(-grid / (2.0 * sigma1 ** 2)).astype(np.float32) * mask
        sk = (sk / sk.sum()).astype(np.float32)
        sk_eff = sk.reshape(NT)
        active = sk_eff > 0.0
        ln_sk = np.where(active, np.log(np.maximum(sk_eff, 1e-30)),
                         np.float32(-100.0)).astype(np.float32)
        n_act = np.float32(active.sum())
        s2c = np.float32(1.0 / (np.sqrt(2.0, dtype=np.float64) * float(sigma2)) / C)
        out.append((ln_sk, n_act, s2c, active))
    return out


def _build(active_taps, n_iter=1):
    from contextlib import ExitStack, nullcontext
    import concourse.tile as tile
    import concourse.bass as bass
    from concourse import bacc, mybir
    from concourse.ap import AP as _AP

    f32 = mybir.dt.float32
    bf16 = mybir.dt.bfloat16
    USE_GPS = True
    AF = mybir.ActivationFunctionType
    AL = mybir.AluOpType
    act = sorted(active_taps)
    act_dis = sorted({t // 5 for t in act})

    nc = bacc.Bacc("TRN2", target_bir_lowering=False, debug=False,
                   num_devices=NCORES)
    xs_d = nc.dram_tensor("xs", [C, SLAB_H, SLAB_W], f32, kind="ExternalInput").ap()
    cst_d = nc.dram_tensor("cst", [128, 64], f32, kind="ExternalInput").ap()
    id_d = nc.dram_tensor("ident", [128, 128], f32, kind="ExternalInput").ap()
    id4_d = nc.dram_tensor("ident4", [128, 4], f32, kind="ExternalInput").ap()
    xbf_d = nc.dram_tensor("xbf", [C, SLAB_H, SLAB_W], bf16, kind="Internal").ap()
    out_d = nc.dram_tensor("out", [C, HALF, W], f32, kind="ExternalOutput").ap()

    # Per-(di) engine split of dj taps: even di -> pair up djs on DVE with
    # one leftover single on GPSIMD; odd di -> one pair on DVE, rest GPSIMD.
    def tap_plan(di, djs):
        npair = len(djs) // 2 if di % 2 == 0 else max(0, len(djs) // 2 - 1)
        pairs, singles, i = [], [], 0
        while npair > 0 and i + 1 < len(djs):
            if djs[i + 1] == djs[i] + 1:
                pairs.append((djs[i], djs[i + 1]))
                i += 2
                npair -= 1
            else:
                singles.append(djs[i])
                i += 1
        singles += djs[i:]
        return pairs, singles

    with tile.TileContext(nc) as tc, ExitStack() as ctx:
        loop_ctx = tc.For_i(0, n_iter, 1) if n_iter > 1 else nullcontext()
        pool_c = ctx.enter_context(tc.tile_pool(name="cstp", bufs=1))
        pool_s = ctx.enter_context(tc.tile_pool(name="stagep", bufs=2))
        pool_x = ctx.enter_context(tc.tile_pool(name="xp", bufs=1))
        pool_m = ctx.enter_context(tc.tile_pool(name="meanp", bufs=1))
        pool_w = ctx.enter_context(tc.tile_pool(name="wp", bufs=1))
        pool_pd = ctx.enter_context(tc.tile_pool(name="prodpd", bufs=2))
        pool_pg = ctx.enter_context(tc.tile_pool(name="prodpg", bufs=3))
        pool_o = ctx.enter_context(tc.tile_pool(name="ogp", bufs=1))
        pool_ps = ctx.enter_context(
            tc.tile_pool(name="psum", bufs=1, space=bass.MemorySpace.PSUM))

        cst = pool_c.tile([128, 64], f32, name="cst")
        nc.sync.dma_start(cst[:], cst_d)
        ident = pool_c.tile([128, 128], f32, name="ident")
        nc.sync.dma_start(ident[:], id_d)
        ident4 = pool_c.tile([128, 4], f32, name="ident4")
        nc.sync.dma_start(ident4[:], id4_d)
        identb = pool_c.tile([128, 128], bf16, name="identb")
        nc.scalar.activation(out=identb[:], in_=ident[:], func=AF.Copy)
        id4b = pool_c.tile([128, 4], bf16, name="id4b")
        nc.scalar.activation(out=id4b[:], in_=ident4[:], func=AF.Copy)
        ctx.enter_context(loop_ctx)

        # ---- chunked fp32 load -> bf16 convert -> channel-sum (PE) ----
        # tail rows 128..132 packed c-major: partition c*4+r, loaded first
        # one untransposed DMA: flat (c, r, w) enumeration lands on
        # partition c*4+r -- c-major packing for free
        xtf = pool_s.tile([128, SLAB_W], f32, name="xtf")
        nc.sync.dma_start(xtf[:], xs_d[:, 128:132, :])
        xtb = pool_x.tile([128, SLAB_W], bf16, name="xtb")
        nc.scalar.activation(out=xtb[:], in_=xtf[:], func=AF.Copy)
        # untransposed tail restage write: dst [C, 4, 260] <- c-major src
        nc.sync.dma_start(xbf_d[:, 128:132, :], xtb[:])
        ps_t = pool_ps.tile([128, 2, W], f32, tag="bk1", name="ps_t")
        ps_tA = _AP(ps_t[:].tensor, ps_t[:].offset,
                    [[ps_t[:].ap[0][0], 4], [1, SLAB_W]])
        nc.tensor.matmul(ps_tA, id4b[:], xtb[:], start=True, stop=True,
                         skip_group_check=True)
        m_sB = pool_m.tile([4, SLAB_W], f32, name="m_sB")
        nc.vector.tensor_scalar_mul(
            out=m_sB[:],
            in0=_AP(ps_t[:].tensor, ps_t[:].offset,
                    [[ps_t[:].ap[0][0], 4], [1, SLAB_W]]),
            scalar1=cst[0:4, 51:52])

        xb = {0: pool_x.tile([128, C, SLAB_W], bf16, name="xb0")}
        ps_m = pool_ps.tile([128, 2, W], f32, tag="bk0", name="ps_m")
        ps_mA = _AP(ps_m[:].tensor, ps_m[:].offset,
                    [list(ps_m[:].ap[0]), [1, SLAB_W]])
        for k in range(C // CHK):
            xst = pool_s.tile([128, CHK, SLAB_W], f32, tag="xst", name=f"xst{k}")
            nc.sync.dma_start(
                xst[:], xs_d[k * CHK:(k + 1) * CHK, 0:128, :].transpose([1, 0, 2]))
            nc.scalar.activation(out=xb[0][:, k * CHK:(k + 1) * CHK, :],
                                 in_=xst[:], func=AF.Copy)
            for c in range(CHK):
                cc = k * CHK + c
                nc.tensor.matmul(ps_mA, identb[:], xb[0][:, cc, :],
                                 start=(cc == 0), stop=(cc == C - 1),
                                 skip_group_check=True)

        # scaled mean m_s = (sum_c x) * s2c   (cst col 51)
        m_sA = pool_m.tile([128, SLAB_W], f32, name="m_sA")
        nc.vector.tensor_scalar_mul(out=m_sA[:], in0=ps_mA, scalar1=cst[:, 51:52])

        # di-shifted views of m_s (rows di..di+128); tails (from early m_sB)
        # first, then mains; msd2 first (every di subtract needs the center)
        msd = {0: m_sA}
        for di in (2, 1, 3, 4):
            t = pool_m.tile([128, SLAB_W], f32, name=f"msd{di}")
            nc.sync.dma_start(t[128 - di:128, :], m_sB[0:di, :])
            msd[di] = t
        for di in (2, 1, 3, 4):
            nc.sync.dma_start(msd[di][0:128 - di, :], m_sA[di:128, :])

        # ---- restage bf16 slab rows 0..128 to DRAM (after msd DMAs) ----
        for k in range(C // CHK):
            nc.sync.dma_start(
                xbf_d[k * CHK:(k + 1) * CHK, 0:128, :].transpose([1, 0, 2]),
                xb[0][:, k * CHK:(k + 1) * CHK, :])

        # ---- shifted bf16 slabs from restaged DRAM ----
        for di in act_dis:
            if di == 0:
                continue
            t = pool_x.tile([128, C, SLAB_W], bf16, name=f"xb{di}")
            nc.sync.dma_start(t[:], xbf_d[:, di:di + 128, :].transpose([1, 0, 2]))
            xb[di] = t

        # ---- phase A: weight planes W_t = Exp(-d^2 + ln sk_t) in bf16 ----
        W5 = pool_c.tile([128, NT, W], bf16, name="W5")
        denom = pool_m.tile([128, W], f32, name="denom")
        first_t = act[0]
        for di in act_dis:
            djs = [dj for dj in range(5) if (di * 5 + dj) in active_taps]
            dj0, ndj = djs[0], len(djs)
            in0 = msd[di][:, dj0:dj0 + W + ndj - 1]
            in0w = _AP(in0.tensor, in0.offset,
                       [list(in0.ap[0]), [1, ndj], [1, W]])
            ctr = msd[2][:, 2:2 + W].unsqueeze(1).broadcast_to([128, ndj, W])
            d5 = pool_w.tile([128, ndj, W], f32, tag="d", name=f"d{di}")
            (nc.gpsimd if USE_GPS else nc.vector).tensor_tensor(out=d5[:], in0=in0w, in1=ctr, op=AL.subtract)
            sq5 = pool_w.tile([128, ndj, W], f32, tag="sq", name=f"sq{di}")
            nc.scalar.activation(out=sq5[:], in_=d5[:], func=AF.Square)
            for j, dj in enumerate(djs):
                t_idx = di * 5 + dj
                nc.scalar.activation(out=W5[:, t_idx, :], in_=sq5[:, j, :],
                                     func=AF.Exp, scale=-1.0,
                                     bias=cst[:, t_idx:t_idx + 1])
                # denominator: sum_t W_t as a GPSIMD chain (SBUF only)
                if t_idx == first_t:
                    (nc.gpsimd if USE_GPS else nc.vector).tensor_copy(denom[:], W5[:, t_idx, :])
                else:
                    (nc.gpsimd if USE_GPS else nc.vector).tensor_tensor(out=denom[:], in0=W5[:, t_idx, :],
                                            in1=denom[:], op=AL.add)
        # recip = 1 / (sum_t W_t + 1e-8*n_active)   (cst col 50)
        dsum = pool_m.tile([128, W], f32, name="dsum")
        nc.vector.tensor_scalar_add(out=dsum[:], in0=denom[:], scalar1=cst[:, 50:51])
        recip = pool_m.tile([128, W], f32, name="recip")
        nc.vector.reciprocal(out=recip[:], in_=dsum[:])

        # ---- phase B: tap MAC; 4 groups x 8ch, psum bank ping-pong.
        # Groups run in pairs (gA even banks 0-3, gB banks 4-7) sweeping di
        # together, so each shifted slab is needed at half the pace and og
        # drains overlap the next pair's matmuls. ----
        GC = 8   # channels per group
        NB = GC // 2  # psum banks per group
        n_taps = len(act)

        def emit_pair_op(g, di, pr):
            ch0 = g * GC
            t0 = di * 5 + pr[0]
            prod = pool_pd.tile([128, 2, GC, W], bf16, tag="pd",
                                name=f"pr{g}_{t0}")
            w2 = W5[:, t0:t0 + 2, :].unsqueeze(2).broadcast_to([128, 2, GC, W])
            xin = xb[di][:, ch0:ch0 + GC, pr[0]:pr[0] + W + 1]
            xw = _AP(xin.tensor, xin.offset,
                     [list(xin.ap[0]), [1, 2], list(xin.ap[1]), [1, W]])
            nc.vector.tensor_tensor(out=prod[:], in0=w2, in1=xw, op=AL.mult)
            return [prod[:, 0, :, :], prod[:, 1, :, :]]

        def emit_single_op(g, di, dj):
            ch0 = g * GC
            t_idx = di * 5 + dj
            prod = pool_pg.tile([128, GC, W], bf16, tag="pg",
                                name=f"pg{g}_{t_idx}")
            (nc.gpsimd if USE_GPS else nc.vector).tensor_tensor(
                out=prod[:],
                in0=W5[:, t_idx, :].unsqueeze(1).broadcast_to([128, GC, W]),
                in1=xb[di][:, ch0:ch0 + GC, dj:dj + W],
                op=AL.mult)
            return [prod[:]]

        for gA in (0, 2):
            gB = gA + 1
            pbs = {g: [pool_ps.tile([128, 2, W], f32,
                                    tag=f"bk{(g % 2) * NB + b}",
                                    name=f"pb{g}_{b}")
                       for b in range(NB)]
                   for g in (gA, gB)}
            done = {gA: 0, gB: 0}

            def accumulate(g, prods):
                for pap in prods:
                    for b in range(NB):
                        nc.tensor.matmul(pbs[g][b][:], identb[:],
                                         pap[:, 2 * b:2 * b + 2, :],
                                         start=(done[g] == 0),
                                         stop=(done[g] == n_taps - 1),
                                         skip_group_check=True)
                    done[g] += 1

            for di in act_dis:
                djs = [dj for dj in range(5) if (di * 5 + dj) in active_taps]
                pairs, singles = tap_plan(di, djs)
                # alternate which group leads on DVE vs GPSIMD per di
                lead, trail = (gA, gB) if di % 2 == 0 else (gB, gA)
                items = []
                for pr in pairs:
                    items.append(("p", lead, pr))
                    items.append(("p", trail, pr))
                for s in singles:
                    items.append(("s", lead, s))
                    items.append(("s", trail, s))
                # interleave DVE-pair ops and GPSIMD-single ops
                p_items = [it for it in items if it[0] == "p"]
                s_items = [it for it in items if it[0] == "s"]
                merged, ip, isg = [], 0, 0
                while ip < len(p_items) or isg < len(s_items):
                    if ip < len(p_items) and (
                            isg >= len(s_items)
                            or ip * max(1, len(s_items)) <=
                            isg * max(1, len(p_items))):
                        merged.append(p_items[ip]); ip += 1
                    else:
                        merged.append(s_items[isg]); isg += 1
                for kind, g, item in merged:
                    if kind == "p":
                        accumulate(g, emit_pair_op(g, di, item))
                    else:
                        accumulate(g, emit_single_op(g, di, item))

            # og = psum * recip on DVE (GPSIMD cannot read PSUM on HW)
            for g in (gA, gB):
                ch0 = g * GC
                og = pool_o.tile([128, GC, W], f32, tag="og", name=f"og{g}")
                for b in range(NB):
                    nc.vector.tensor_tensor(
                        out=og[:, 2 * b:2 * b + 2, :], in0=pbs[g][b][:],
                        in1=recip[:].unsqueeze(1).broadcast_to([128, 2, W]),
                        op=AL.mult)
                nc.sync.dma_start(
                    out_d[ch0:ch0 + GC, :, :].transpose([1, 0, 2]), og[:])

    nc.compile()
    return nc


def _build2(tapsA, tapsB, n_iter=1):
    """Balanced resharding: each core runs unit A (k5 sample-half, 16ch,
    tapsA) + unit B (k3 sample-half, 16ch, tapsB). Identical SPMD shape."""
    from contextlib import ExitStack, nullcontext
    import concourse.tile as tile
    import concourse.bass as bass
    from concourse import bacc, mybir
    from concourse.ap import AP as _AP

    f32 = mybir.dt.float32
    f32r = mybir.dt.float32r
    bf16 = mybir.dt.bfloat16
    AF = mybir.ActivationFunctionType
    AL = mybir.AluOpType
    OC = 16   # own channels per unit
    GC = 8    # channels per psum group
    NB = GC // 2

    nc = bacc.Bacc("TRN2", target_bir_lowering=False, debug=False,
                   num_devices=NCORES)
    d = {}
    for u in ("A", "B"):
        d[u] = {
            "xs": nc.dram_tensor(f"xs{u}", [C, SLAB_H, SLAB_W], f32,
                                 kind="ExternalInput").ap(),
            "cst": nc.dram_tensor(f"cst{u}", [128, 64], f32,
                                  kind="ExternalInput").ap(),
            "xbf": nc.dram_tensor(f"xbf{u}", [OC, SLAB_H, SLAB_W], bf16,
                                  kind="Internal").ap(),
            "out": nc.dram_tensor(f"out{u}", [OC, HALF, W], f32,
                                  kind="ExternalOutput").ap(),
            "taps": sorted(tapsA if u == "A" else tapsB),
        }
        d[u]["dis"] = sorted({t // 5 for t in d[u]["taps"]})
    id_d = nc.dram_tensor("ident", [128, 128], f32, kind="ExternalInput").ap()
    id4_d = nc.dram_tensor("ident4", [128, 4], f32, kind="ExternalInput").ap()

    def tap_plan(di, djs):
        if len(djs) <= 3:
            npair = len(djs) // 2
        else:
            npair = len(djs) // 2 if di % 2 == 0 else len(djs) // 2 - 1
        pairs, singles, i = [], [], 0
        while npair > 0 and i + 1 < len(djs):
            if djs[i + 1] == djs[i] + 1:
                pairs.append((djs[i], djs[i + 1]))
                i += 2
                npair -= 1
            else:
                singles.append(djs[i])
                i += 1
        singles += djs[i:]
        return pairs, singles

    with tile.TileContext(nc) as tc, ExitStack() as ctx:
        loop_ctx = tc.For_i(0, n_iter, 1) if n_iter > 1 else nullcontext()
        pool_c = ctx.enter_context(tc.tile_pool(name="cstp", bufs=1))
        pool_s = ctx.enter_context(tc.tile_pool(name="stagep", bufs=2))
        pool_x = ctx.enter_context(tc.tile_pool(name="xp", bufs=1))
        pool_m = ctx.enter_context(tc.tile_pool(name="meanp", bufs=1))
        pool_w = ctx.enter_context(tc.tile_pool(name="wp", bufs=1))
        pool_pd = ctx.enter_context(tc.tile_pool(name="prodpd", bufs=3))
        pool_pg = ctx.enter_context(tc.tile_pool(name="prodpg", bufs=3))
        pool_o = ctx.enter_context(tc.tile_pool(name="ogp", bufs=2))
        pool_ps = ctx.enter_context(
            tc.tile_pool(name="psum", bufs=1, space=bass.MemorySpace.PSUM))

        ident = pool_c.tile([128, 128], f32, name="ident")
        nc.sync.dma_start(ident[:], id_d)
        ident4 = pool_c.tile([128, 4], f32, name="ident4")
        nc.sync.dma_start(ident4[:], id4_d)
        identb = pool_c.tile([128, 128], bf16, name="identb")
        nc.scalar.activation(out=identb[:], in_=ident[:], func=AF.Copy)
        id4b = pool_c.tile([128, 4], bf16, name="id4b")
        nc.scalar.activation(out=id4b[:], in_=ident4[:], func=AF.Copy)
        for u in ("A", "B"):
            t = pool_c.tile([128, 64], f32, name=f"cstt{u}")
            nc.sync.dma_start(t[:], d[u]["cst"])
            d[u]["cstt"] = t
        ctx.enter_context(loop_ctx)

        # ---- tails for both units (c-major pack), tail means on PE early ----
        for u in ("A", "B"):
            e = d[u]
            xtf = pool_s.tile([128, SLAB_W], f32, name=f"xtf{u}")
            nc.sync.dma_start(xtf[:], e["xs"][:, 128:132, :])
            xtb = pool_x.tile([128, SLAB_W], bf16, name=f"xtb{u}")
            nc.scalar.activation(out=xtb[:], in_=xtf[:], func=AF.Copy)
            # restage only own-16 tail rows: [16, 4, 260] <- partitions 0:64
            nc.sync.dma_start(e["xbf"][:, 128:132, :], xtb[0:64, :])
            ps_t = pool_ps.tile([128, 2, W], f32, tag="bk1", name=f"ps_t{u}")
            ps_tA = _AP(ps_t[:].tensor, ps_t[:].offset,
                        [[ps_t[:].ap[0][0], 4], [1, SLAB_W]])
            nc.tensor.matmul(ps_tA, id4b[:], xtb[:], start=True, stop=True,
                             skip_group_check=True)
            msB = pool_m.tile([4, SLAB_W], f32, name=f"m_sB{u}")
            nc.vector.tensor_scalar_mul(out=msB[:], in0=ps_tA,
                                        scalar1=e["cstt"][0:4, 51:52])
            e["xtb"], e["m_sB"] = xtb, msB

        # ---- unit A: stage chunks, chsum on PE (own bf16 + other f32r) ----
        eA = d["A"]
        xb0A = pool_x.tile([128, OC, SLAB_W], bf16, name="xb0A")
        ps_m = pool_ps.tile([128, 2, W], f32, tag="bk0", name="ps_mA")
        ps_mA = _AP(ps_m[:].tensor, ps_m[:].offset,
                    [list(ps_m[:].ap[0]), [1, SLAB_W]])
        # mean-only chunks (4..7) load FIRST: their fp32 chsum matmuls
        # have no convert dependency and pipeline behind the DMAs; own
        # chunks (0..3) follow, converting to bf16 on ACT as they land.
        nk = C // CHK
        xsts = {}
        for k in list(range(nk // 2, nk)) + list(range(nk // 2)):
            xst = pool_s.tile([128, CHK, SLAB_W], f32, tag="xst", bufs=4,
                              name=f"xsA{k}")
            nc.sync.dma_start(
                xst[:], eA["xs"][k * CHK:(k + 1) * CHK, 0:128, :]
                .transpose([1, 0, 2]))
            xsts[k] = xst
            if k < nk // 2:
                nc.scalar.activation(out=xb0A[:, k * CHK:(k + 1) * CHK, :],
                                     in_=xsts[k][:], func=AF.Copy)
                for c in range(CHK):
                    cc = k * CHK + c
                    nc.tensor.matmul(ps_mA, identb[:], xb0A[:, cc, :],
                                     start=False, stop=(cc == OC - 1),
                                     skip_group_check=True)
            else:
                for c in range(CHK):
                    cc = k * CHK + c
                    nc.tensor.matmul(ps_mA, ident[:], xsts[k][:, c, :],
                                     start=(cc == OC), stop=False,
                                     skip_group_check=True)
        m_sAA = pool_m.tile([128, SLAB_W], f32, name="m_sAA")
        nc.vector.tensor_scalar_mul(out=m_sAA[:], in0=ps_mA,
                                    scalar1=eA["cstt"][:, 51:52])
        eA["xb"] = {0: xb0A}
        eA["m_sA"] = m_sAA

        def emit_msd(e):
            msd = {0: e["m_sA"]}
            dis = sorted(set(e["dis"]) | {2})
            order = [2] + [x for x in dis if x != 2 and x != 0]
            for di in order:
                t = pool_m.tile([128, SLAB_W], f32, name=f"msd{di}{id(e) % 97}")
                nc.sync.dma_start(t[128 - di:128, :], e["m_sB"][0:di, :])
                msd[di] = t
            for di in order:
                nc.sync.dma_start(msd[di][0:128 - di, :], e["m_sA"][di:128, :])
            e["msd"] = msd

        def emit_restage_main(e, xb0):
            nc.sync.dma_start(
                e["xbf"][:, 0:128, :].transpose([1, 0, 2]), xb0[:])

        def emit_shifts(e):
            for di in e["dis"]:
                if di == 0:
                    continue
                t = pool_x.tile([128, OC, SLAB_W], bf16,
                                name=f"xb{di}{id(e) % 97}")
                nc.sync.dma_start(
                    t[:], e["xbf"][:, di:di + 128, :].transpose([1, 0, 2]))
                e["xb"][di] = t

        def emit_phaseA(e):
            pfx = "A" if e is eA else "B"
            W5 = pool_c.tile([128, NT, W], bf16, name=f"W5{pfx}")
            denom = pool_m.tile([128, W], f32, name=f"den{pfx}")
            taps = set(e["taps"])
            first_t = min(e["taps"])
            for di in e["dis"]:
                djs = [dj for dj in range(5) if (di * 5 + dj) in taps]
                dj0, ndj = djs[0], len(djs)
                in0 = e["msd"][di][:, dj0:dj0 + W + ndj - 1]
                in0w = _AP(in0.tensor, in0.offset,
                           [list(in0.ap[0]), [1, ndj], [1, W]])
                ctr = e["msd"][2][:, 2:2 + W].unsqueeze(1) \
                    .broadcast_to([128, ndj, W])
                d5 = pool_w.tile([128, ndj, W], f32, tag="d", name=f"d{pfx}{di}")
                nc.gpsimd.tensor_tensor(out=d5[:], in0=in0w, in1=ctr,
                                        op=AL.subtract)
                sq5 = pool_w.tile([128, ndj, W], f32, tag="sq",
                                  name=f"sq{pfx}{di}")
                nc.scalar.activation(out=sq5[:], in_=d5[:], func=AF.Square)
                for j, dj in enumerate(djs):
                    t_idx = di * 5 + dj
                    nc.scalar.activation(out=W5[:, t_idx, :], in_=sq5[:, j, :],
                                         func=AF.Exp, scale=-1.0,
                                         bias=e["cstt"][:, t_idx:t_idx + 1])
                    if t_idx == first_t:
                        nc.gpsimd.tensor_copy(denom[:], W5[:, t_idx, :])
                    else:
                        nc.gpsimd.tensor_tensor(out=denom[:],
                                                in0=W5[:, t_idx, :],
                                                in1=denom[:], op=AL.add)
            e["W5"], e["denom"] = W5, denom

        def emit_recip(e):
            pfx = "A" if e is eA else "B"
            dsum = pool_m.tile([128, W], f32, name=f"dsum{pfx}")
            nc.vector.tensor_scalar_add(out=dsum[:], in0=e["denom"][:],
                                        scalar1=e["cstt"][:, 50:51])
            recip = pool_m.tile([128, W], f32, name=f"recip{pfx}")
            nc.vector.reciprocal(out=recip[:], in_=dsum[:])
            e["recip"] = recip

        def emit_phaseB(e, di_subset, pbs, done):
            taps = set(e["taps"])
            pfx = "A" if e is eA else "B"
            n_taps = len(e["taps"])
            for di in di_subset:
                djs = [dj for dj in range(5) if (di * 5 + dj) in taps]
                pairs, singles = tap_plan(di, djs)
                lead, trail = ((0, 1) if di % 2 == 0 else (1, 0))
                items = []
                for pr in pairs:
                    items.append(("p", lead, pr))
                    items.append(("p", trail, pr))
                for sg in singles:
                    items.append(("s", lead, sg))
                    items.append(("s", trail, sg))
                p_items = [it for it in items if it[0] == "p"]
                s_items = [it for it in items if it[0] == "s"]
                merged, ip, isg = [], 0, 0
                while ip < len(p_items) or isg < len(s_items):
                    if ip < len(p_items) and (
                            isg >= len(s_items)
                            or ip * max(1, len(s_items)) <=
                            isg * max(1, len(p_items))):
                        merged.append(p_items[ip]); ip += 1
                    else:
                        merged.append(s_items[isg]); isg += 1
                for kind, g, item in merged:
                    ch0 = g * GC
                    if kind == "p":
                        t0 = di * 5 + item[0]
                        prod = pool_pd.tile([128, 2, GC, W], bf16, tag="pd",
                                            name=f"pr{pfx}{g}_{t0}")
                        w2 = e["W5"][:, t0:t0 + 2, :].unsqueeze(2) \
                            .broadcast_to([128, 2, GC, W])
                        xin = e["xb"][di][:, ch0:ch0 + GC,
                                          item[0]:item[0] + W + 1]
                        xw = _AP(xin.tensor, xin.offset,
                                 [list(xin.ap[0]), [1, 2],
                                  list(xin.ap[1]), [1, W]])
                        nc.vector.tensor_tensor(out=prod[:], in0=w2, in1=xw,
                                                op=AL.mult)
                        paps = [prod[:, 0, :, :], prod[:, 1, :, :]]
                    else:
                        t_idx = di * 5 + item
                        prod = pool_pg.tile([128, GC, W], bf16, tag="pg",
                                            name=f"pg{pfx}{g}_{t_idx}")
                        nc.gpsimd.tensor_tensor(
                            out=prod[:],
                            in0=e["W5"][:, t_idx, :].unsqueeze(1)
                            .broadcast_to([128, GC, W]),
                            in1=e["xb"][di][:, ch0:ch0 + GC,
                                            item:item + W],
                            op=AL.mult)
                        paps = [prod[:]]
                    for pap in paps:
                        for b in range(NB):
                            nc.tensor.matmul(pbs[g][b][:], identb[:],
                                             pap[:, 2 * b:2 * b + 2, :],
                                             start=(done[g] == 0),
                                             stop=(done[g] == n_taps - 1),
                                             skip_group_check=True)
                        done[g] += 1

        def emit_og_drain(e, pbs):
            # ACT drains PSUM -> SBUF (frees banks without touching DVE)
            pfx = "A" if e is eA else "B"
            raws = {}
            for g in (0, 1):
                ogr = pool_o.tile([128, GC, W], bf16, tag="ogr",
                                  name=f"ogr{pfx}{g}")
                for b in range(NB):
                    nc.scalar.activation(out=ogr[:, 2 * b:2 * b + 2, :],
                                         in_=pbs[g][b][:], func=AF.Copy)
                raws[g] = ogr
            return raws

        def emit_og_out(e, raws):
            # divide on GPSIMD (SBUF-only: legal), then DMA out
            pfx = "A" if e is eA else "B"
            for g in (0, 1):
                ch0 = g * GC
                og = pool_o.tile([128, GC, W], f32, tag="og", bufs=1,
                                 name=f"og{pfx}{g}")
                for b in range(NB):
                    nc.gpsimd.tensor_tensor(
                        out=og[:, 2 * b:2 * b + 2, :],
                        in0=raws[g][:, 2 * b:2 * b + 2, :],
                        in1=e["recip"][:].unsqueeze(1)
                        .broadcast_to([128, 2, W]),
                        op=AL.mult)
                for hh in range(2):
                    c0 = ch0 + hh * (GC // 2)
                    nc.sync.dma_start(
                        e["out"][c0:c0 + GC // 2, :, :].transpose([1, 0, 2]),
                        og[:, hh * (GC // 2):(hh + 1) * (GC // 2), :])

        def mk_pbs(pfx):
            return {g: [pool_ps.tile([128, 2, W], f32, tag=f"bk{g * NB + b}",
                                     name=f"pb{pfx}{g}_{b}")
                        for b in range(NB)]
                    for g in (0, 1)}

        # A: mean shifts, restage, phase A
        emit_msd(eA)
        emit_restage_main(eA, xb0A)
        emit_shifts(eA)
        emit_phaseA(eA)

        # A phase B part 1 (first 3 dis)
        pbsA = mk_pbs("A")
        doneA = {0: 0, 1: 0}
        disA = eA["dis"]
        emit_phaseB(eA, disA[:3], pbsA, doneA)

        # B: stage chunks + DVE-reduce mean (no PSUM; banks are A's)
        eB = d["B"]
        xb0B = pool_x.tile([128, OC, SLAB_W], bf16, name="xb0B")
        macc = pool_m.tile([128, SLAB_W], f32, name="maccB")
        mpart = pool_m.tile([128, SLAB_W], f32, name="mpartB")
        for k in range(C // CHK):
            xst = pool_s.tile([128, CHK, SLAB_W], f32, tag="xst", bufs=4, name=f"xsB{k}")
            nc.sync.dma_start(
                xst[:], eB["xs"][k * CHK:(k + 1) * CHK, 0:128, :]
                .transpose([1, 0, 2]))
            own = k * CHK < OC
            if own:
                nc.scalar.activation(out=xb0B[:, k * CHK:(k + 1) * CHK, :],
                                     in_=xst[:], func=AF.Copy)
            # reduce chunk over channels: in AP ordered (w, c)
            xv = xst[:]
            chsum_in = _AP(xv.tensor, xv.offset,
                           [list(xv.ap[0]), [1, SLAB_W], [SLAB_W, CHK]])
            tgt = macc if k == 0 else mpart
            nc.vector.tensor_reduce(out=tgt[:], in_=chsum_in,
                                    axis=mybir.AxisListType.X, op=AL.add)
            if k > 0:
                nc.vector.tensor_tensor(out=macc[:], in0=mpart[:],
                                        in1=macc[:], op=AL.add)
        m_sAB = pool_m.tile([128, SLAB_W], f32, name="m_sAB")
        nc.vector.tensor_scalar_mul(out=m_sAB[:], in0=macc[:],
                                    scalar1=eB["cstt"][:, 51:52])
        eB["xb"] = {0: xb0B}
        eB["m_sA"] = m_sAB
        emit_msd(eB)
        emit_restage_main(eB, xb0B)
        emit_shifts(eB)

        # B phase A early (Pool/ACT prep while DVE/PE finish A)
        emit_phaseA(eB)

        # A phase B rest; drain A's banks on ACT so B can start
        emit_phaseB(eA, disA[3:], pbsA, doneA)
        emit_recip(eA)
        rawsA = emit_og_drain(eA, pbsA)

        # B phase B; A's divide+out runs on GPSIMD/SP underneath
        pbsB = mk_pbs("B")
        doneB = {0: 0, 1: 0}
        emit_phaseB(eB, eB["dis"], pbsB, doneB)
        emit_og_out(eA, rawsA)
        emit_recip(eB)
        rawsB = emit_og_drain(eB, pbsB)
        emit_og_out(eB, rawsB)

    nc.compile()
    return nc


def _prep_inputs(x, params):
    """Build per-core in_maps."""
    x = np.ascontiguousarray(x, dtype=np.float32)
    tap_consts = _host_tap_constants(params)
    active = set()
    for (ln_sk, n_act, s2c, act_mask) in tap_consts:
        active |= {t for t in range(NT) if act_mask[t]}
    xp = np.pad(x, ((0, 0), (0, 0), (2, 2), (2, 2)))
    in_maps = []
    for core in range(NCORES):
        b, half = core // 2, core % 2
        h0 = half * HALF
        slab = np.ascontiguousarray(xp[b, :, h0:h0 + SLAB_H, :])
        ln_sk, n_act, s2c, _ = tap_consts[b]
        cst = np.zeros((128, 64), np.float32)
        cst[:, 0:25] = ln_sk[None, :]
        cst[:, 50] = np.float32(1e-8) * n_act
        cst[:, 51] = s2c
        in_maps.append({"xs": slab, "cst": cst,
                        "ident": np.eye(128, dtype=np.float32),
                        "ident4": np.tile(np.eye(4, dtype=np.float32), (32, 1))})
    return in_maps, frozenset(active)


def _prep_inputs2(x, params, tap_consts, k5s, k3s):
    """Per-core in_maps for the balanced 2-unit path."""
    xp = np.pad(x, ((0, 0), (0, 0), (2, 2), (2, 2)))
    unitsA = [(s, h, cg) for s in k5s for h in (0, 1) for cg in (0, 1)]
    unitsB = [(s, h, cg) for s in k3s for h in (0, 1) for cg in (0, 1)]
    in_maps, placements = [], []
    ident = np.eye(128, dtype=np.float32)
    ident4 = np.tile(np.eye(4, dtype=np.float32), (32, 1))
    for core in range(NCORES):
        im = {"ident": ident, "ident4": ident4}
        plc = {}
        for u, (s, h, cg) in (("A", unitsA[core]), ("B", unitsB[core])):
            own = list(range(cg * 16, cg * 16 + 16))
            rest = [c for c in range(C) if c not in own]
            slab = np.ascontiguousarray(
                xp[s, own + rest, h * HALF:h * HALF + SLAB_H, :])
            ln_sk, n_act, s2c, _ = tap_consts[s]
            cst = np.zeros((128, 64), np.float32)
            cst[:, 0:25] = ln_sk[None, :]
            cst[:, 50] = np.float32(1e-8) * n_act
            cst[:, 51] = s2c
            im[f"xs{u}"] = slab
            im[f"cst{u}"] = cst
            plc[u] = (s, h, own)
        in_maps.append(im)
        placements.append(plc)
    return in_maps, placements


def kernel(x, params, n_iter=1, **_):
    from concourse.bass_utils import run_bass_kernel_spmd
    x = np.ascontiguousarray(x, dtype=np.float32)
    tap_consts = _host_tap_constants(params)
    k5s = [b for b in range(B) if tap_consts[b][3].sum() == 25]
    k3s = [b for b in range(B) if b not in k5s]
    if len(k5s) == 2 and len(k3s) == 2:
        tapsA = frozenset(
            t for b in k5s for t in range(NT) if tap_consts[b][3][t])
        tapsB = frozenset(
            t for b in k3s for t in range(NT) if tap_consts[b][3][t])
        key = ("nc2", tapsA, tapsB, n_iter)
        if key not in _CACHE:
            _CACHE[key] = _build2(tapsA, tapsB, n_iter)
        nc = _CACHE[key]
        in_maps, placements = _prep_inputs2(x, params, tap_consts, k5s, k3s)
        res = run_bass_kernel_spmd(nc, in_maps, list(range(NCORES)))
        out = np.empty((B, C, H, W), np.float32)
        for core in range(NCORES):
            for u in ("A", "B"):
                s, h, own = placements[core][u]
                out[s, own, h * HALF:(h + 1) * HALF, :] = \
                    res.results[core][f"out{u}"]
        return out
    # fallback: uniform union-tap path
    in_maps, active = _prep_inputs(x, params)
    key = ("nc", active, n_iter)
    if key not in _CACHE:
        _CACHE[key] = _build(active, n_iter)
    nc = _CACHE[key]
    res = run_bass_kernel_spmd(nc, in_maps, list(range(NCORES)))
    out = np.empty((B, C, H, W), np.float32)
    for core in range(NCORES):
        b, half = core // 2, core % 2
        out[b, :, half * HALF:(half + 1) * HALF, :] = res.results[core]["out"]
    return out
